# revision 74
# baseline (speedup 1.0000x reference)
"""Trainium2 Bass kernel for nn_LocalAggregator (GNN message passing).

Computation (reference semantics):
    te    = p0*exp(-t) + p1
    h     = [hidden[..., :127] | te]
    e_k   = leaky_relu((h*a_k) @ h^T, 0.2)          k = 0..3
    alpha = softmax(select_by_adj(e_k, adj, -inf))   over last axis
    out   = alpha @ h

Device strategy (pure data-parallel over batch, 8 cores x 8 batches):
  - Score planes e_k are symmetric bilinear forms, computed in [j, i]
    layout (neighbor j on partitions).  The softmax denominator (sum
    over j) falls out of a ones-stationary matmul -- no transposes, no
    partition reductions.
  - hTk planes are packed (0,2,1,3) so the PSUM pair tiles are
    D=[e0|e2] and S=[e1|e3]; masks/kill planes ship per-jc as
    [b0|b0|b1n|zkill].
  - The 4-way adjacency select runs as a copy_predicated ladder over the
    PSUM score planes using host-shipped uint8 bit-plane masks
    (b0 = lsb(adj-1), b1n = adj<3).  The final stage selects INTO e2
    with the complement mask so e0's PSUM bank frees at mux time.
  - adj==0 kill: a 0/-1024 fp8e5 mask plane is accumulated into e0's
    PSUM slice by an identity-stationary matmul BEFORE the mux (adj==0
    routes to e0 through the ladder, so the bias survives selection and
    exp underflows to 0).  This keeps the Pool engine off the
    mux->prelu->exp critical chain and halves the kill-plane DMA.  The
    identity is built on-device (memset + affine_select).
  - The PE p-state ramp (0.65/1.2GHz until 3us of continuous busy) is
    burned off by a warm-up matmul chain on memset data while the first
    input DMAs are still in flight.
  - Prelu runs per-jc (frees PSUM banks early); Exp runs jc-pair-merged
    straight out of the prelu tile.
  - A batch's aggregation is deferred into the NEXT batch and split in
    two dep-pinned phases (feature matmuls in the jc0/jc1 mux window,
    denominator matmuls + evac/DMA in the jc3 window) so no single
    deprioritized PE burst exceeds the slack between the eS-refill
    matmuls that gate the DVE mux.  The last batch's aggregation chases
    its own per-jc exps to shorten the drain.
  - PSUM evacuation runs on Act; output DMAs issue from the Act HWDGE
    ring while input DMAs issue from SP, keeping every sequencer below
    the DVE mux cadence (the DVE copy_predicated ladder, 7.4us/batch,
    is the steady-state bottleneck).
  - The raw aggregation (bf16, + f32 denominator) is DMA'd out; the
    softmax division happens on the host.
"""

import os
import sys

import numpy as np

for _p in ("/opt/trn_rl_repo", "/root/.axon_site/_ro/trn_rl_repo"):
    if os.path.isdir(_p) and _p not in sys.path:
        sys.path.insert(0, _p)

B, N, DIM = 64, 512, 128
NCORES = 8
BPC = B // NCORES          # batches per core
JC = N // 128              # j-chunks per batch
HAUG = 132                 # 128 dims + ones col + pad
LEAKY_ALPHA = 0.2
ZKILL = -1024.0
NWARM = 3                  # p-state warm-up matmuls

# blob A (per batch): [ hT (4*128) | hTk{0,2,1,3} (4*512) ] bf16.
# The first DMA carries only [hT | hTk0 | hTk2] (all stationaries plus
# what jc0's D-pair matmuls need) so the pipeline ramp is short and no
# score matmul ever waits on the late haug blob.
AW = 4 * 128 + 4 * N
A1W = 128 + 2 * N          # [hT_jc0 | hTk0 | hTk2]
# blob C (late DMA, aggregation-only): [ haug (JC*HAUG) ] bf16
OFF_HAUG = 0
CW = JC * HAUG
# u8 blob: per-jc [ b0 | b0 | b1n | zkill(fp8e5) ], each N cols
MMJC = 4 * N
MMW = JC * MMJC

_CACHE = {}


def _build_nc(repeat=1):
    import concourse.bass as bass
    from concourse import bacc, mybir
    from concourse.tile import TileContext

    bf16 = mybir.dt.bfloat16
    f32 = mybir.dt.float32
    u8 = mybir.dt.uint8
    fp8 = mybir.dt.float8e5
    act = mybir.ActivationFunctionType

    nc = bacc.Bacc(None, target_bir_lowering=False)

    bina_d = nc.declare_dram_parameter("bina", [BPC, 128, AW], bf16, isOutput=False)
    mm_d = nc.declare_dram_parameter("mm", [BPC, 128, MMW], u8, isOutput=False)
    binc_d = nc.declare_dram_parameter("binc", [BPC, 128, CW], bf16, isOutput=False)
    out_d = nc.declare_dram_parameter("out", [BPC, 128, N], bf16, isOutput=True)
    z_d = nc.declare_dram_parameter("z", [BPC, 1, N], f32, isOutput=True)

    with TileContext(nc) as tc:
        with (
            tc.tile_pool(name="inp", bufs=5) as inp,
            tc.tile_pool(name="work", bufs=3) as work,
            tc.tile_pool(name="npool", bufs=5) as npool,
            tc.tile_pool(name="outp", bufs=3) as outp,
            tc.tile_pool(name="constp", bufs=1) as constp,
            tc.tile_pool(name="pseD", bufs=2, space=bass.MemorySpace.PSUM) as pseD,
            tc.tile_pool(name="pseS", bufs=1, space=bass.MemorySpace.PSUM) as pseS,
            tc.tile_pool(name="psa", bufs=1, space=bass.MemorySpace.PSUM) as psa,
            tc.tile_pool(name="psz", bufs=1, space=bass.MemorySpace.PSUM) as psz,
        ):
            # constants: ones block (col 0 doubles as the z-matmul
            # stationary), identity for the kill accumulation, and a warm-up
            # moving tile.  All built on-device -- no DMA on the ramp.
            ones_t = constp.tile([128, 128], bf16, tag="ones", name="ones")
            nc.gpsimd.memset(ones_t[:], 1.0)
            ident_t = constp.tile([128, 128], bf16, tag="ident", name="ident")
            nc.gpsimd.affine_select(
                ident_t[:], ones_t[:], [[1, 128]],
                mybir.AluOpType.is_equal, 0.0,
                base=0, channel_multiplier=-1,
            )
            warm_t = constp.tile([128, N], bf16, tag="warm", name="warm")
            nc.gpsimd.memset(warm_t[:], 0.0)
            # p-state warm-up: keep the PE continuously busy from ~1us so the
            # first real score matmuls run at full clock.  Rides the psz pool
            # so no extra PSUM bank is needed.
            wps = psz.tile([1, N], f32, tag="zps", name="warm_ps")
            for _ in range(NWARM):
                nc.tensor.matmul(wps[:], ones_t[:, 0:1], warm_t[:],
                                 start=True, stop=True)

            pending = []    # [(uid, b, npairs, binc_t)] awaiting aggregation

            def agg_tiles(uid):
                aggT = psa.tile([128, N], f32, tag="aggT", name=f"aggT{uid}")
                zps = psz.tile([1, N], f32, tag="zps", name=f"zps{uid}")
                return aggT, zps

            def agg_mms(aggT, zps, npairs, binc_t, jcc_range):
                for jcc in jcc_range:
                    nplane = npairs[jcc // 2][:, (jcc % 2) * N:
                                              (jcc % 2 + 1) * N]
                    nc.tensor.matmul(
                        aggT[:],
                        binc_t[:, OFF_HAUG + jcc * HAUG:
                               OFF_HAUG + jcc * HAUG + DIM],
                        nplane,
                        start=(jcc == 0),
                        stop=(jcc == JC - 1),
                    )
                    nc.tensor.matmul(
                        zps[:], ones_t[:, 0:1], nplane,
                        start=(jcc == 0),
                        stop=(jcc == JC - 1),
                    )

            def agg_out(uid, b, aggT, zps):
                # evacuate PSUM (DMA/GPSIMD cannot read it); divide on host.
                # Output DMAs go out the Act HWDGE ring.  The last batch's
                # denominator evacuation uses the (then-idle) DVE so it
                # overlaps the Act copy on the drain chain.
                outt = outp.tile([128, N], bf16, tag="outt", name=f"outt{uid}")
                zout = outp.tile([1, N], f32, tag="zout", name=f"zout{uid}")
                if uid == BPC - 1:
                    # drain chain: z rides the (then-idle) DVE and goes out
                    # the SP ring so its issue overlaps the Act-ring out DMA.
                    nc.scalar.copy(outt[:], aggT[:])
                    nc.scalar.dma_start(out=out_d[b], in_=outt[:])
                    nc.vector.tensor_copy(zout[:], zps[:])
                    nc.sync.dma_start(out=z_d[b], in_=zout[:])
                else:
                    nc.scalar.copy(outt[:], aggT[:])
                    nc.scalar.copy(zout[:], zps[:])
                    nc.scalar.dma_start(out=out_d[b], in_=outt[:])
                    nc.scalar.dma_start(out=z_d[b], in_=zout[:])

            def emit_agg(uid, b, npairs, binc_t, dep=None):
                # Aggregation phase, deprioritized and -- via a 1-element
                # dummy Act copy from the NEXT batch's first npre tile (value
                # immediately overwritten by start=True) -- dep-pinned behind
                # the next batch's first prelu, so its matmuls never occupy
                # the PE stream slots that the next units' score matmuls
                # (which gate the DVE mux) need.
                #   aggT[d, i] = sum_j haug[j, d] * n[j, i]
                #   z[i]      = sum_j n[j, i]        (softmax denominator)
                save_prio = tc.cur_priority
                tc.cur_priority = save_prio + 500
                aggT, zps = agg_tiles(uid)
                if dep is not None:
                    nc.scalar.copy(aggT[0:1, 0:1], dep)
                    nc.scalar.copy(zps[0:1, 0:1], dep)
                agg_mms(aggT, zps, npairs, binc_t, range(JC))
                agg_out(uid, b, aggT, zps)
                tc.cur_priority = save_prio

            def emit_agg_phase1(uid, b, npairs, binc_t, npre_t):
                # feature-aggregation half of the deferred batch, spread over
                # TWO of the next batch's mux windows so no single PE burst
                # exceeds the slack between eS refills: jcc{0,1} are pinned
                # on the first prelu (jc0/jc1 window); jcc{2,3} read their
                # haug stationaries through a Pool bypass-copy that "reads"
                # the second prelu's output, delaying them into the jc2
                # window without an illegal mid-group PSUM write.
                save_prio = tc.cur_priority
                tc.cur_priority = save_prio + 500
                aggT, zps = agg_tiles(uid)
                nc.scalar.copy(aggT[0:1, 0:1], npre_t[0:1, 0:1])
                for jcc in range(JC):
                    nplane = npairs[jcc // 2][:, (jcc % 2) * N:
                                              (jcc % 2 + 1) * N]
                    nc.tensor.matmul(
                        aggT[:],
                        binc_t[:, OFF_HAUG + jcc * HAUG:
                               OFF_HAUG + jcc * HAUG + DIM],
                        nplane,
                        start=(jcc == 0),
                        stop=(jcc == JC - 1),
                    )
                tc.cur_priority = save_prio
                return aggT, zps

            def emit_agg_phase2(uid, b, npairs, aggT, zps, dep):
                # denominator half + evac/DMA: pinned into the next batch's
                # jc3 window (the only PE-slack window), keeping the burst
                # that collides with the jc2/jc3 eS refills small.
                save_prio = tc.cur_priority
                tc.cur_priority = save_prio + 500
                nc.scalar.copy(zps[0:1, 0:1], dep)
                for jcc in range(JC):
                    nplane = npairs[jcc // 2][:, (jcc % 2) * N:
                                              (jcc % 2 + 1) * N]
                    nc.tensor.matmul(
                        zps[:], ones_t[:, 0:1], nplane,
                        start=(jcc == 0),
                        stop=(jcc == JC - 1),
                    )
                agg_out(uid, b, aggT, zps)
                tc.cur_priority = save_prio

            for rep, b in [(r, bb) for r in range(repeat) for bb in range(BPC)]:
                uid = rep * BPC + b
                last = uid == repeat * BPC - 1
                bina_t = inp.tile([128, AW], bf16, tag="bina")
                mm0_t = inp.tile([128, MMJC], u8, tag="mm0")
                mm1_t = inp.tile([128, MMJC], u8, tag="mm1")
                mmr_t = inp.tile([128, (JC - 2) * MMJC], u8, tag="mmr")
                binc_t = inp.tile([128, CW], bf16, tag="binc")
                nc.sync.dma_start(out=bina_t[:, 0:A1W], in_=bina_d[b, :, 0:A1W])
                nc.sync.dma_start(out=bina_t[:, A1W:], in_=bina_d[b, :, A1W:])
                # jc0's kill plane (N fp8 cols) rides ahead of the masks so
                # the Z matmul (PE side, +pipeline+sem) clears before the
                # mux's own mask operand even lands.
                nc.sync.dma_start(out=mm0_t[:, 3 * N:], in_=mm_d[b, :, 3 * N:MMJC])
                nc.sync.dma_start(out=mm0_t[:, 0:3 * N], in_=mm_d[b, :, 0:3 * N])
                nc.sync.dma_start(out=mm1_t[:], in_=mm_d[b, :, MMJC:2 * MMJC])
                nc.sync.dma_start(out=mmr_t[:], in_=mm_d[b, :, 2 * MMJC:])
                nc.sync.dma_start(out=binc_t[:], in_=binc_d[b])

                def stat_hT(jc):
                    if jc == 0:
                        return bina_t[:, 0:128]
                    return bina_t[:, A1W + (jc - 1) * 128:A1W + jc * 128]

                if last:
                    laggT, lzps = None, None
                    # flush the deferred batch up front: the PE has slack
                    # here, and its evac/DMA must stay off this batch's
                    # Act-side drain chain (also keeps PSUM pool cycling in
                    # program order w.r.t. our own agg tiles below).
                    while pending:
                        emit_agg(*pending.pop(0))

                npairs = []
                deferred = None
                for jc in range(JC):
                    if jc == 0:
                        mmt, off = mm0_t, 0
                    elif jc == 1:
                        mmt, off = mm1_t, 0
                    else:
                        mmt, off = mmr_t, (jc - 2) * MMJC
                    b00m = mmt[:, off:off + 2 * N]
                    b1nm = mmt[:, off + 2 * N:off + 3 * N]
                    zmov = mmt[:, off + 3 * N:off + 4 * N].bitcast(fp8)

                    # pair-tile PSUM layout: D = [e0 | e2], S = [e1 | e3]
                    # (each plane exactly one bank) so mux stage 1 is a
                    # single flat [128, 2N] copy_predicated.  Each pair is
                    # one [128,2N]-moving matmul; e0 additionally accumulates
                    # the 0/-1024 kill plane via the identity stationary.
                    eD = pseD.tile([128, 2 * N], f32, tag="eD", name=f"eD{uid}_{jc}")
                    eS = pseS.tile([128, 2 * N], f32, tag="eS", name=f"eS{uid}_{jc}")
                    nc.tensor.matmul(
                        eD[:, 0:N], stat_hT(jc), bina_t[:, 128:128 + N],
                        start=True, stop=False,
                    )
                    nc.tensor.matmul(
                        eD[:, N:2 * N], stat_hT(jc), bina_t[:, 128 + N:128 + 2 * N],
                        start=True, stop=True,
                    )
                    nc.tensor.matmul(
                        eS[:, 0:N], stat_hT(jc), bina_t[:, A1W + 384:A1W + 384 + N],
                        start=True, stop=True,
                    )
                    nc.tensor.matmul(
                        eS[:, N:2 * N], stat_hT(jc), bina_t[:, A1W + 384 + N:A1W + 384 + 2 * N],
                        start=True, stop=True,
                    )
                    nc.tensor.matmul(
                        eD[:, 0:N], ident_t[:], zmov,
                        start=False, stop=True,
                    )
                    if jc == 1 and pending:
                        deferred = pending.pop(0)
                        dtiles = emit_agg_phase1(*deferred, npre_t=npre)
                    if jc == 3 and deferred is not None:
                        emit_agg_phase2(deferred[0], deferred[1], deferred[2],
                                        *dtiles, dep=npre[0:1, 0:1])
                        deferred = None

                    # 4-way select ladder -> D[N:2N] holds e_{adj-1}:
                    # stage 1 muxes both pairs in one op (mask [b0|b0]),
                    # stage 2 selects into the e2 slot with ~b1.  Stage 2 is
                    # nudged behind the NEXT chunk's stage 1 (priority +1) so
                    # the DVE chews it while the single-buffered eS bank is
                    # refilled by the PE -- software pipelining that hides the
                    # ~0.8us eS turnaround.
                    nc.vector.copy_predicated(eD[:], b00m, eS[:])
                    nc.vector.copy_predicated(eD[:, N:2 * N], b1nm, eD[:, 0:N])

                    if jc % 2 == 0:
                        npre = work.tile([128, 2 * N], bf16, tag="npre",
                                         name=f"npre{uid}_{jc // 2}")
                        npair = npool.tile([128, 2 * N], bf16, tag="npair",
                                           name=f"npair{uid}_{jc // 2}")
                        npairs.append(npair)
                    nc.scalar.activation(
                        npre[:, (jc % 2) * N:(jc % 2 + 1) * N], eD[:, N:2 * N],
                        act.Prelu, alpha=LEAKY_ALPHA
                    )

                    if last:
                        # drain-friendly: per-jc exp, with the aggregation
                        # chain chasing each chunk immediately.
                        nc.scalar.activation(
                            npair[:, (jc % 2) * N:(jc % 2 + 1) * N],
                            npre[:, (jc % 2) * N:(jc % 2 + 1) * N], act.Exp)
                        save_prio = tc.cur_priority
                        tc.cur_priority = save_prio + 500
                        if laggT is None:
                            laggT, lzps = agg_tiles(uid)
                        agg_mms(laggT, lzps, npairs, binc_t, range(jc, jc + 1))
                        if jc == JC - 1:
                            agg_out(uid, b, laggT, lzps)
                        tc.cur_priority = save_prio
                    elif jc % 2 == 1:
                        # jc-pair-merged exp
                        nc.scalar.activation(npair[:], npre[:], act.Exp)

                if not last:
                    pending.append((uid, b, npairs, binc_t))

            for args in pending:
                emit_agg(*args)

    nc.compile()
    return nc


def _get_nc():
    if "nc" not in _CACHE:
        _CACHE["nc"] = _build_nc()
    return _CACHE["nc"]


def _host_prep(hidden, adj, input_times, a0, a1, a2, a3, p0, p1):
    import ml_dtypes

    bf16 = ml_dtypes.bfloat16
    fp8 = ml_dtypes.float8_e5m2

    hidden = np.asarray(hidden, dtype=np.float32)
    adj = np.asarray(adj)
    input_times = np.asarray(input_times, dtype=np.float32)

    te = np.asarray(p0, np.float32) * np.exp(-input_times) + np.asarray(p1, np.float32)
    h = np.concatenate([hidden[:, :, :-1], te[:, :, None]], axis=2)      # [B,N,128] f32

    hT = np.swapaxes(h, 1, 2)                                            # [B,128,N]
    A = np.stack([a0, a1, a2, a3], 0).astype(np.float32)                 # [4,128]
    hTk = A[None, :, :, None] * hT[:, None, :, :]                        # [B,4,128,N]

    bina = np.zeros((B, 128, AW), bf16)
    bina[:, :, 0:128] = hT[:, :, 0:128].astype(bf16)
    bina[:, :, 128:128 + N] = hTk[:, 0].astype(bf16)
    bina[:, :, 128 + N:128 + 2 * N] = hTk[:, 2].astype(bf16)
    bina[:, :, A1W:A1W + 384] = hT[:, :, 128:].astype(bf16)
    bina[:, :, A1W + 384:A1W + 384 + N] = hTk[:, 1].astype(bf16)
    bina[:, :, A1W + 384 + N:] = hTk[:, 3].astype(bf16)

    binc = np.zeros((B, 128, CW), bf16)

    # haug[b, jp, jc, c] = h[b, jc*128+jp, c] (+ ones col)
    haug = np.zeros((B, N, HAUG), np.float32)
    haug[:, :, :DIM] = h
    haug[:, :, DIM] = 1.0
    haug = haug.reshape(B, JC, 128, HAUG).transpose(0, 2, 1, 3)
    binc[:, :, OFF_HAUG:] = haug.reshape(B, 128, JC * HAUG).astype(bf16)

    def chunkT(m):
        # mask[b, i, j] -> transposed + chunked [b, jp, jc, i]
        mT = np.swapaxes(m, 1, 2)
        return mT.reshape(B, JC, 128, N).transpose(0, 2, 1, 3)   # [B,128,JC,N]

    # per-jc mask blob: [b0 | b0 | b1n | zkill] (b0 duplicated so mux
    # stage 1 reads one contiguous [128, 2N] mask)
    mmb = np.zeros((B, 128, JC, 4, N), np.uint8)
    b0 = chunkT((((adj - 1) & 1) * (adj > 0)).astype(np.uint8))
    b1n = chunkT((adj < 3).astype(np.uint8))
    zk = chunkT(np.where(adj == 0, np.float32(ZKILL), np.float32(0.0))
                ).astype(fp8).view(np.uint8)
    mmb[:, :, :, 0, :] = b0
    mmb[:, :, :, 1, :] = b0
    mmb[:, :, :, 2, :] = b1n
    mmb[:, :, :, 3, :] = zk
    mmb = mmb.reshape(B, 128, MMW)

    in_maps = []
    for c in range(NCORES):
        s = slice(c * BPC, (c + 1) * BPC)
        in_maps.append({"bina": bina[s], "binc": binc[s], "mm": mmb[s]})
    return in_maps


def run(inputs, trace=False, **spmd_kwargs):
    """Full pipeline; returns (output, BassKernelResults)."""
    from concourse import bass_utils

    in_maps = _host_prep(**inputs)
    nc = _get_nc()
    res = bass_utils.run_bass_kernel_spmd(
        nc, in_maps, core_ids=list(range(NCORES)), trace=trace, **spmd_kwargs
    )
    outs = []
    for r in res.results:
        o = np.asarray(r["out"]).astype(np.float32)   # [BPC, 128(d), N(i)] bf16
        z = np.asarray(r["z"], np.float32).reshape(BPC, 1, N)
        normed = o / z                                # softmax divide on host
        outs.append(normed.transpose(0, 2, 1))        # -> [BPC, N(i), 128(d)]
    full = np.concatenate(outs, axis=0)
    return full, res


def kernel(**inputs) -> np.ndarray:
    out, _ = run(inputs, trace=False)
    return out
